# revision 35
# baseline (speedup 1.0000x reference)
"""Trainium2 Bass kernel for a dense GAT layer (B=4, N=2048, FIN=128, K=4 heads, D=32).

Math (per batch b):
    Wh = (H @ W).reshape(N, K, D)
    s[i,k] = <Wh[i,k,:], a_src[k,:]>;  t[j,k] = <Wh[j,k,:], a_dst[k,:]>
    e[i,j,k] = leaky_relu(s[i,k] + t[j,k], 0.2), masked to -inf where A[i,j] == 0
    alpha = softmax_j(e);  out[i] = sum_j alpha[i,j,k] * Wh[j,k,:]

Kernel reformulation (exact in exact arithmetic):
    exp(lrelu(x)) = max(exp(x), exp(0.2 x)); with x = s_i + t_j both branches are
    rank-1, and the i-side factor exp(0.2 s_i) cancels in the softmax. So with
    G_i = exp(0.8 s_i), H_j = exp(0.8 t_j), F2_j = exp(0.2 t_j), m = (A > 0):
        w[j,i]   = m[i,j] * max(G_i * H_j, 1)
        out[i,:] = (sum_j w[j,i] * F2_j * Wh[j,:]) / (sum_j w[j,i] * F2_j)
    Scores live in transposed [j (partitions), i (free)] layout so the
    j-contraction runs on the tensor engine with PSUM accumulation; appending F2
    as an extra column of the stationary operand yields the denominators free.

Everything on the score path is bf16 (tolerance is 2e-2): the PE runs matmuls at
1 cycle/row instead of fp32's 4, the DVE hits its 4x (tensor_scalar) and 2x
(tensor_tensor) perf modes, and the mask ships from the host as a ready-made
bf16 0/1 tile so no compare pass runs on device.

Structure: a short PSUM-staging prologue (WhT, s/t scores, G broadcast) is
wrapped in its own pool and released so the main loop can hold 8 PSUM
accumulator banks (4 heads x 2 i-halves) and sweep each j-chunk exactly once
with full-width (FD 1024/2048) elementwise ops. The G broadcast rides a PE
selector matmul (no DRAM bounce); whf tiles are built via the DMA xbar
transpose, interleaved with the mask stream on the one SP DGE queue so neither
starves. The mask multiply is split DVE/Pool to balance the two engines.

Sharding: 8 cores = 4 batches x 2 row-halves. The host rotates each core's H
rows / A columns so its own query rows are always local rows 0..1023 (keeps the
SPMD program identical across cores), and ships H and the mask pre-transposed so
the device needs no fp32 transposes for them.
"""

import numpy as np
from contextlib import ExitStack

import concourse.bacc as bacc
import concourse.mybir as mybir
import concourse.tile as tile
from concourse.bass_utils import run_bass_kernel_spmd

B, N, FIN = 4, 2048, 128
KH, DH = 4, 32
P = 128
NI = 1024  # query rows per core
JT = N // P  # 16 j-chunks
NIB = 2  # output i-halves (PSUM accumulation width)
IBS = NI // NIB  # 512

f32 = mybir.dt.float32
bf16 = mybir.dt.bfloat16

_CACHE = {}


def _build_program():
    nc = bacc.Bacc("TRN2", target_bir_lowering=False, debug=False)

    def din(name, shape, dtype=f32):
        return nc.dram_tensor(name, list(shape), dtype, kind="ExternalInput").ap()

    wu_d = din("wu", (P, 8 + P), bf16)   # tiny tensor: first DMA, PE warm-up fodder
    mT_d = din("maskT", (N, NI), bf16)   # 0/1 mask slab transposed: [j, i]
    sel_d = din("sel", (KH, KH * P), bf16)  # head-selector for the G broadcast
    CPA = 2 * KH + NI        # [WS | WD | HT own rows]
    CPB = P + P + NI         # [W | ident | HT other rows]
    cpackA_d = din("cpackA", (P, CPA), bf16)
    cpackB_d = din("cpackB", (P, CPB), bf16)
    oaux_d = nc.dram_tensor(
        "oaux", [NIB, KH, DH + 1, IBS], f32, kind="ExternalOutput"
    ).ap()

    Exp = mybir.ActivationFunctionType.Exp
    Copy = mybir.ActivationFunctionType.Copy
    MULT = mybir.AluOpType.mult
    MAX = mybir.AluOpType.max

    with tile.TileContext(nc) as tc, ExitStack() as ctx:
        const = ctx.enter_context(tc.tile_pool(name="const", bufs=1))
        big = ctx.enter_context(tc.tile_pool(name="big", bufs=1))
        dbuf = ctx.enter_context(tc.tile_pool(name="dbuf", bufs=2))
        work = ctx.enter_context(tc.tile_pool(name="work", bufs=6))

        # ---- constants / inputs ----
        cpackA = const.tile([P, CPA], bf16, tag="cpackA")
        nc.sync.dma_start(cpackA[:], cpackA_d[:])
        wu = const.tile([P, 8 + P], bf16, tag="wu")
        nc.sync.dma_start(wu[:], wu_d[:])
        sel = const.tile([KH, KH * P], bf16, tag="sel")
        nc.sync.dma_start(sel[:], sel_d[:])
        cpackB = const.tile([P, CPB], bf16, tag="cpackB")
        nc.sync.dma_start(cpackB[:], cpackB_d[:])
        sWS = cpackA[:, 0:KH]             # W @ blockdiag(a_src): [fin, k]
        sWD = cpackA[:, KH:2 * KH]        # W @ blockdiag(a_dst): [fin, k]
        HT0 = cpackA[:, 2 * KH:]          # [fin, own rows 0:NI]
        sbW = cpackB[:, 0:P]
        HT1 = cpackB[:, 2 * P:]           # [fin, rows NI:N]

        def HTc(jt):  # HT j-chunk jt
            c = (jt % 8) * P
            return HT0[:, c:c + P] if jt < 8 else HT1[:, c:c + P]

        WhT = big.tile([P, N], bf16, tag="WhT")  # [kd, n] = (H @ W).T
        Grow = big.tile([KH, NI], bf16, tag="Grow")  # exp(0.8 s)
        Gbm = dbuf.tile([P, KH, NI], bf16, tag="Gbm", bufs=1)
        Hcol = big.tile([P, JT, KH], f32, tag="Hcol")    # exp(0.8 t)
        F2col = big.tile([P, JT, KH], f32, tag="F2col")  # exp(0.2 t)

        # whf/mask stream on the SP DGE queue, interleaved so the xbar
        # transposes (whf source) and the mask tiles arrive in lock-step with
        # per-j-chunk consumption
        mTs, wrows = [], []
        for jt in range(JT):
            mt = dbuf.tile([P, NI], bf16, tag="mTp", bufs=JT, name=f"mT{jt}")
            wr = dbuf.tile([P, P], bf16, tag="wrow", bufs=JT, name=f"wrow{jt}")
            mTs.append(mt)
            wrows.append(wr)

        # ---- prologue A: everything that needs PSUM staging, then release ----
        with tc.tile_pool(name="ps", bufs=2, space="PSUM") as ps:
            # PE warm-up: junk transposes so the HAM clock ramps while the
            # input DMAs are still in flight
            for _ in range(14):
                pwu = ps.tile([P, 512], bf16, tag="stgb", bufs=2)
                nc.tensor.transpose(pwu[0:8, 0:P], wu[:, 0:8], wu[:, 8:8 + P])

            # WhT chunks 0-1 feed the s-scores; chunks 2-3 are evacuated
            # later (after the G chain) but share the two staging bufs
            def wht_chunk(q, eng):
                pw = ps.tile([P, 512], f32, tag="stg", bufs=2, name=f"pw{q}")
                ht = HT0 if q < 2 else HT1
                nc.tensor.matmul(pw[:], sbW,
                                 ht[:, (q % 2) * 512:(q % 2 + 1) * 512],
                                 start=True, stop=True)
                nc.scalar.copy(WhT[:, q * 512:(q + 1) * 512], pw[:])

            # s-scores straight from HT (no WhT dependency); 512-wide
            # matmuls so each PSUM output stays within one bank
            for q in range(NI // 512):
                ps3 = ps.tile([P, 512], f32, tag="stg3", bufs=1)
                nc.tensor.matmul(ps3[0:KH, :], sWS,
                                 HT0[:, q * 512:(q + 1) * 512],
                                 start=True, stop=True)
                nc.scalar.activation(Grow[:, q * 512:(q + 1) * 512],
                                     ps3[0:KH, :], Exp, scale=0.8)

            # G broadcast on the PE: Gbm[:, k, i] = Grow[k, i] everywhere,
            # via a one-hot selector stationary (evacuated per half-head)
            def g_bcast(k):
                for h in range(2):
                    pg = ps.tile([P, IBS], f32, tag="pg", bufs=2)
                    nc.tensor.matmul(pg[:], sel[:, k * P:(k + 1) * P],
                                     Grow[:, h * IBS:(h + 1) * IBS],
                                     start=True, stop=True)
                    nc.scalar.copy(Gbm[:, k, h * IBS:(h + 1) * IBS], pg[:])

            # t-scores straight from HT chunks (no WhT dependency); the
            # first half only needs cpackA, so Hcol for j-chunks 0-7 clears
            # ACT before the bulk input even lands
            tcat = ps.tile([P, JT * KH], f32, tag="tcat", bufs=1)
            halfJ = JT // 2 * KH
            for jt in range(JT // 2):
                nc.tensor.matmul(tcat[:, jt * KH:(jt + 1) * KH],
                                 HTc(jt), sWD, start=True, stop=True)
            g_bcast(0)
            nc.scalar.activation(
                Hcol[:, 0:JT // 2].rearrange("p j k -> p (j k)"),
                tcat[:, 0:halfJ], Exp, scale=0.8)
            g_bcast(1)
            nc.scalar.activation(
                F2col[:, 0:JT // 2].rearrange("p j k -> p (j k)"),
                tcat[:, 0:halfJ], Exp, scale=0.2)
            for jt in range(JT // 2, JT):
                nc.tensor.matmul(tcat[:, jt * KH:(jt + 1) * KH],
                                 HTc(jt), sWD, start=True, stop=True)
            g_bcast(2)
            nc.scalar.activation(
                Hcol[:, JT // 2:].rearrange("p j k -> p (j k)"),
                tcat[:, halfJ:], Exp, scale=0.8)
            nc.scalar.activation(
                F2col[:, JT // 2:].rearrange("p j k -> p (j k)"),
                tcat[:, halfJ:], Exp, scale=0.2)
            g_bcast(3)
            for q in range(4):
                wht_chunk(q, "act")

        # interleaved whf-transpose / mask stream (SP queue, paced with use)
        for jt in range(JT):
            nc.sync.dma_start_transpose(wrows[jt][:],
                                        WhT[:, jt * P:(jt + 1) * P])
            nc.sync.dma_start(mTs[jt][:], mT_d[jt * P:(jt + 1) * P, :])

        # whf[jt][:, k, :] = [Wh_k * F2 | F2] via ACT scale-copies
        whf = []
        for jt in range(JT):
            wt = big.tile([P, KH, DH + 1], bf16, tag=f"whf{jt}", name=f"whf{jt}")
            for k in range(KH):
                nc.scalar.activation(
                    wt[:, k, 0:DH], wrows[jt][:, k * DH:(k + 1) * DH], Copy,
                    scale=F2col[:, jt, k:k + 1],
                )
            nc.scalar.copy(wt[:, :, DH:DH + 1], F2col[:, jt, :, None])
            whf.append(wt)

        # ---- main loop: 8 PSUM accumulator banks, one sweep over j-chunks ----
        pspv = ctx.enter_context(tc.tile_pool(name="pspv", bufs=1, space="PSUM"))
        pv = [[pspv.tile([DH + 1, IBS], f32, tag=f"pv{k}{h}", name=f"pv{k}{h}")
               for h in range(NIB)] for k in range(KH)]

        for jt in range(JT):
            y = work.tile([P, KH, NI], bf16, tag="y")
            for k in range(KH):
                nc.vector.tensor_scalar(
                    y[:, k, :], Gbm[:, k, :], Hcol[:, jt, k:k + 1], 1.0,
                    MULT, MAX,
                )
            # mask multiply, split for DVE/Pool balance: DVE takes heads 0+1
            # (one broadcast op); Pool takes head 3 and every other j-chunk
            # also head 2, except the final two j-chunks (all-DVE so the
            # slower Pool ops never gate the drain into the epilogue).
            nc.vector.tensor_mul(
                y[:, 0:2, :], y[:, 0:2, :],
                mTs[jt][:, None, :].broadcast_to((P, 2, NI)))
            tail = jt >= JT - 3
            k2eng = nc.gpsimd if (jt % 2 == 0 and not tail) else nc.vector
            k3eng = nc.vector if tail else nc.gpsimd
            k2eng.tensor_mul(y[:, 2, :], y[:, 2, :], mTs[jt][:, :])
            k3eng.tensor_mul(y[:, 3, :], y[:, 3, :], mTs[jt][:, :])
            for k in range(KH):
                for h in range(NIB):
                    nc.tensor.matmul(
                        pv[k][h][:],
                        whf[jt][:, k, :],
                        y[:, k, h * IBS:(h + 1) * IBS],
                        start=(jt == 0),
                        stop=(jt == JT - 1),
                    )

        # epilogue: raw [33, i] accumulators (numerators + denominator row)
        # through SBUF (DMA cannot read PSUM), copies split ACT/DVE; the
        # host divides/un-transposes
        for h in range(NIB):
            otT = work.tile([DH + 1, KH, IBS], f32, tag="otT", bufs=2,
                            name=f"otT{h}")
            for k in range(KH):
                if k % 2 == 0:
                    nc.scalar.copy(otT[:, k, :], pv[k][h][:])
                else:
                    nc.vector.tensor_copy(otT[:, k, :], pv[k][h][:])
            dq = nc.sync if h == 0 else nc.scalar
            dq.dma_start(oaux_d[h].rearrange("k d i -> d k i"), otT[:])

    nc.compile()
    return nc


def _host_prep(H, A, W, a_src, a_dst):
    """Build the 8 per-core input maps (layout/dtype prep only)."""
    import ml_dtypes
    bf = ml_dtypes.bfloat16

    Ssrc = np.zeros((FIN, KH), np.float32)
    Sdst = np.zeros((FIN, KH), np.float32)
    for k in range(KH):
        Ssrc[k * DH:(k + 1) * DH, k] = a_src[k]
        Sdst[k * DH:(k + 1) * DH, k] = a_dst[k]
    WS = (W.astype(np.float32) @ Ssrc)  # fold W into the score vectors
    WD = (W.astype(np.float32) @ Sdst)

    sel = np.zeros((KH, KH * P), np.float32)
    for k in range(KH):
        sel[k, k * P:(k + 1) * P] = 1.0
    sel = sel.astype(bf)
    wu_host = np.ones((P, 8 + P), np.float32)
    wu_host[:, 8:] = np.eye(P, dtype=np.float32)
    wu_host = wu_host.astype(bf)

    maskB = (A > 0)  # [B, N, N] bool

    in_maps = []
    for c in range(8):
        b, half = divmod(c, 2)
        i0 = half * NI
        HbT = np.roll(H[b], -i0, axis=0).T
        maskT = np.ascontiguousarray(
            np.roll(maskB[b, i0:i0 + NI, :], -i0, axis=1).T
        ).astype(bf)
        cpackA = np.concatenate([WS, WD, HbT[:, 0:NI]], axis=1).astype(bf)
        cpackB = np.concatenate(
            [W.astype(np.float32), np.eye(P, dtype=np.float32), HbT[:, NI:]],
            axis=1,
        ).astype(bf)
        in_maps.append({
            "maskT": maskT,
            "cpackA": np.ascontiguousarray(cpackA),
            "cpackB": np.ascontiguousarray(cpackB),
            "wu": wu_host,
            "sel": sel,
        })
    return in_maps


def kernel(H, A, W, a_src, a_dst, _want_results=False, _trace=False):
    H = np.asarray(H); A = np.asarray(A); W = np.asarray(W)
    a_src = np.asarray(a_src); a_dst = np.asarray(a_dst)

    if "nc" not in _CACHE:
        _CACHE["nc"] = _build_program()
    nc = _CACHE["nc"]

    in_maps = _host_prep(H, A, W, a_src, a_dst)
    res = run_bass_kernel_spmd(nc, in_maps, list(range(8)), trace=_trace)

    out = np.empty((B, N, KH * DH), np.float32)
    for c in range(8):
        b, half = divmod(c, 2)
        i0 = half * NI
        aux = res.results[c]["oaux"]  # [NIB, KH, DH+1, IBS]
        slab = aux[:, :, 0:DH, :] / aux[:, :, DH:DH + 1, :]
        # [h, k, d, i] -> rows (h*IBS + i), cols (k*DH + d)
        out[b, i0:i0 + NI, :] = (
            slab.transpose(0, 3, 1, 2).reshape(NI, KH * DH)
        )
    if _want_results:
        return out, res
    return out


# revision 36
# speedup vs baseline: 1.0101x; 1.0101x over previous
"""Trainium2 Bass kernel for a dense GAT layer (B=4, N=2048, FIN=128, K=4 heads, D=32).

Math (per batch b):
    Wh = (H @ W).reshape(N, K, D)
    s[i,k] = <Wh[i,k,:], a_src[k,:]>;  t[j,k] = <Wh[j,k,:], a_dst[k,:]>
    e[i,j,k] = leaky_relu(s[i,k] + t[j,k], 0.2), masked to -inf where A[i,j] == 0
    alpha = softmax_j(e);  out[i] = sum_j alpha[i,j,k] * Wh[j,k,:]

Kernel reformulation (exact in exact arithmetic):
    exp(lrelu(x)) = max(exp(x), exp(0.2 x)); with x = s_i + t_j both branches are
    rank-1, and the i-side factor exp(0.2 s_i) cancels in the softmax. So with
    G_i = exp(0.8 s_i), H_j = exp(0.8 t_j), F2_j = exp(0.2 t_j), m = (A > 0):
        w[j,i]   = m[i,j] * max(G_i * H_j, 1)
        out[i,:] = (sum_j w[j,i] * F2_j * Wh[j,:]) / (sum_j w[j,i] * F2_j)
    Scores live in transposed [j (partitions), i (free)] layout so the
    j-contraction runs on the tensor engine with PSUM accumulation; appending F2
    as an extra column of the stationary operand yields the denominators free.

Everything on the score path is bf16 (tolerance is 2e-2): the PE runs matmuls at
1 cycle/row instead of fp32's 4, the DVE hits its 4x (tensor_scalar) and 2x
(tensor_tensor) perf modes, and the mask ships from the host as a ready-made
bf16 0/1 tile so no compare pass runs on device.

Structure: a short PSUM-staging prologue (WhT, s/t scores, G broadcast) is
wrapped in its own pool and released so the main loop can hold 8 PSUM
accumulator banks (4 heads x 2 i-halves) and sweep each j-chunk exactly once
with full-width (FD 1024/2048) elementwise ops. The G broadcast rides a PE
selector matmul (no DRAM bounce); whf tiles are built via the DMA xbar
transpose, interleaved with the mask stream on the one SP DGE queue so neither
starves. The mask multiply is split DVE/Pool to balance the two engines.

Sharding: 8 cores = 4 batches x 2 row-halves. The host rotates each core's H
rows / A columns so its own query rows are always local rows 0..1023 (keeps the
SPMD program identical across cores), and ships H and the mask pre-transposed so
the device needs no fp32 transposes for them.
"""

import numpy as np
from contextlib import ExitStack

import concourse.bacc as bacc
import concourse.mybir as mybir
import concourse.tile as tile
from concourse.bass_utils import run_bass_kernel_spmd

B, N, FIN = 4, 2048, 128
KH, DH = 4, 32
P = 128
NI = 1024  # query rows per core
JT = N // P  # 16 j-chunks
NIB = 2  # output i-halves (PSUM accumulation width)
IBS = NI // NIB  # 512

f32 = mybir.dt.float32
bf16 = mybir.dt.bfloat16

_CACHE = {}


def _build_program():
    nc = bacc.Bacc("TRN2", target_bir_lowering=False, debug=False)

    def din(name, shape, dtype=f32):
        return nc.dram_tensor(name, list(shape), dtype, kind="ExternalInput").ap()

    wu_d = din("wu", (P, 8 + P), bf16)   # tiny tensor: first DMA, PE warm-up fodder
    mT_d = din("maskT", (N, NI), bf16)   # 0/1 mask slab transposed: [j, i]
    sel_d = din("sel", (KH, KH * P), bf16)  # head-selector for the G broadcast
    CPA = 2 * KH + NI        # [WS | WD | HT own rows]
    CPB = P + P + NI         # [W | ident | HT other rows]
    cpackA_d = din("cpackA", (P, CPA), bf16)
    cpackB_d = din("cpackB", (P, CPB), bf16)
    oaux_d = nc.dram_tensor(
        "oaux", [NIB, KH, DH + 1, IBS], f32, kind="ExternalOutput"
    ).ap()

    Exp = mybir.ActivationFunctionType.Exp
    Copy = mybir.ActivationFunctionType.Copy
    MULT = mybir.AluOpType.mult
    MAX = mybir.AluOpType.max

    with tile.TileContext(nc) as tc, ExitStack() as ctx:
        const = ctx.enter_context(tc.tile_pool(name="const", bufs=1))
        big = ctx.enter_context(tc.tile_pool(name="big", bufs=1))
        dbuf = ctx.enter_context(tc.tile_pool(name="dbuf", bufs=2))
        work = ctx.enter_context(tc.tile_pool(name="work", bufs=6))

        # ---- constants / inputs ----
        wu = const.tile([P, 8 + P], bf16, tag="wu")
        nc.sync.dma_start(wu[:], wu_d[:])
        cpackA = const.tile([P, CPA], bf16, tag="cpackA")
        nc.sync.dma_start(cpackA[:], cpackA_d[:])
        sel = const.tile([KH, KH * P], bf16, tag="sel")
        nc.sync.dma_start(sel[:], sel_d[:])
        cpackB = const.tile([P, CPB], bf16, tag="cpackB")
        nc.sync.dma_start(cpackB[:], cpackB_d[:])
        sWS = cpackA[:, 0:KH]             # W @ blockdiag(a_src): [fin, k]
        sWD = cpackA[:, KH:2 * KH]        # W @ blockdiag(a_dst): [fin, k]
        HT0 = cpackA[:, 2 * KH:]          # [fin, own rows 0:NI]
        sbW = cpackB[:, 0:P]
        HT1 = cpackB[:, 2 * P:]           # [fin, rows NI:N]

        def HTc(jt):  # HT j-chunk jt
            c = (jt % 8) * P
            return HT0[:, c:c + P] if jt < 8 else HT1[:, c:c + P]

        WhT = big.tile([P, N], bf16, tag="WhT")  # [kd, n] = (H @ W).T
        Grow = big.tile([KH, NI], bf16, tag="Grow")  # exp(0.8 s)
        Gbm = dbuf.tile([P, KH, NI], bf16, tag="Gbm", bufs=1)
        Hcol = big.tile([P, JT, KH], f32, tag="Hcol")    # exp(0.8 t)
        F2col = big.tile([P, JT, KH], f32, tag="F2col")  # exp(0.2 t)

        # whf/mask stream on the SP DGE queue, interleaved so the xbar
        # transposes (whf source) and the mask tiles arrive in lock-step with
        # per-j-chunk consumption
        mTs, wrows = [], []
        for jt in range(JT):
            mt = dbuf.tile([P, NI], bf16, tag="mTp", bufs=JT, name=f"mT{jt}")
            wr = dbuf.tile([P, P], bf16, tag="wrow", bufs=JT, name=f"wrow{jt}")
            mTs.append(mt)
            wrows.append(wr)

        # ---- prologue A: everything that needs PSUM staging, then release ----
        with tc.tile_pool(name="ps", bufs=2, space="PSUM") as ps:
            # PE warm-up: junk transposes so the HAM clock ramps while the
            # input DMAs are still in flight
            for _ in range(14):
                pwu = ps.tile([P, 512], bf16, tag="stgb", bufs=2)
                nc.tensor.transpose(pwu[0:8, 0:P], wu[:, 0:8], wu[:, 8:8 + P])

            # WhT chunks 0-1 feed the s-scores; chunks 2-3 are evacuated
            # later (after the G chain) but share the two staging bufs
            def wht_chunk(q, eng):
                pw = ps.tile([P, 512], f32, tag="stg", bufs=2, name=f"pw{q}")
                ht = HT0 if q < 2 else HT1
                nc.tensor.matmul(pw[:], sbW,
                                 ht[:, (q % 2) * 512:(q % 2 + 1) * 512],
                                 start=True, stop=True)
                nc.scalar.copy(WhT[:, q * 512:(q + 1) * 512], pw[:])

            # s-scores straight from HT (no WhT dependency); 512-wide
            # matmuls so each PSUM output stays within one bank
            for q in range(NI // 512):
                ps3 = ps.tile([P, 512], f32, tag="stg3", bufs=1)
                nc.tensor.matmul(ps3[0:KH, :], sWS,
                                 HT0[:, q * 512:(q + 1) * 512],
                                 start=True, stop=True)
                nc.scalar.activation(Grow[:, q * 512:(q + 1) * 512],
                                     ps3[0:KH, :], Exp, scale=0.8)

            # G broadcast on the PE: Gbm[:, k, i] = Grow[k, i] everywhere,
            # via a one-hot selector stationary (evacuated per half-head)
            def g_bcast(k):
                for h in range(2):
                    pg = ps.tile([P, IBS], f32, tag="pg", bufs=2)
                    nc.tensor.matmul(pg[:], sel[:, k * P:(k + 1) * P],
                                     Grow[:, h * IBS:(h + 1) * IBS],
                                     start=True, stop=True)
                    nc.scalar.copy(Gbm[:, k, h * IBS:(h + 1) * IBS], pg[:])

            # t-scores straight from HT chunks (no WhT dependency); the
            # first half only needs cpackA, so Hcol for j-chunks 0-7 clears
            # ACT before the bulk input even lands
            tcat = ps.tile([P, JT * KH], f32, tag="tcat", bufs=1)
            halfJ = JT // 2 * KH
            for jt in range(JT // 2):
                nc.tensor.matmul(tcat[:, jt * KH:(jt + 1) * KH],
                                 HTc(jt), sWD, start=True, stop=True)
            g_bcast(0)
            nc.scalar.activation(
                Hcol[:, 0:JT // 2].rearrange("p j k -> p (j k)"),
                tcat[:, 0:halfJ], Exp, scale=0.8)
            g_bcast(1)
            nc.scalar.activation(
                F2col[:, 0:JT // 2].rearrange("p j k -> p (j k)"),
                tcat[:, 0:halfJ], Exp, scale=0.2)
            for jt in range(JT // 2, JT):
                nc.tensor.matmul(tcat[:, jt * KH:(jt + 1) * KH],
                                 HTc(jt), sWD, start=True, stop=True)
            g_bcast(2)
            nc.scalar.activation(
                Hcol[:, JT // 2:].rearrange("p j k -> p (j k)"),
                tcat[:, halfJ:], Exp, scale=0.8)
            nc.scalar.activation(
                F2col[:, JT // 2:].rearrange("p j k -> p (j k)"),
                tcat[:, halfJ:], Exp, scale=0.2)
            g_bcast(3)
            for q in range(4):
                wht_chunk(q, "act")

        # interleaved whf-transpose / mask stream (SP queue, paced with use)
        for jt in range(JT):
            nc.sync.dma_start_transpose(wrows[jt][:],
                                        WhT[:, jt * P:(jt + 1) * P])
            nc.sync.dma_start(mTs[jt][:], mT_d[jt * P:(jt + 1) * P, :])

        # whf[jt][:, k, :] = [Wh_k * F2 | F2] via ACT scale-copies
        whf = []
        for jt in range(JT):
            wt = big.tile([P, KH, DH + 1], bf16, tag=f"whf{jt}", name=f"whf{jt}")
            for k in range(KH):
                nc.scalar.activation(
                    wt[:, k, 0:DH], wrows[jt][:, k * DH:(k + 1) * DH], Copy,
                    scale=F2col[:, jt, k:k + 1],
                )
            nc.scalar.copy(wt[:, :, DH:DH + 1], F2col[:, jt, :, None])
            whf.append(wt)

        # ---- main loop: 8 PSUM accumulator banks, one sweep over j-chunks ----
        pspv = ctx.enter_context(tc.tile_pool(name="pspv", bufs=1, space="PSUM"))
        pv = [[pspv.tile([DH + 1, IBS], f32, tag=f"pv{k}{h}", name=f"pv{k}{h}")
               for h in range(NIB)] for k in range(KH)]

        for jt in range(JT):
            y = work.tile([P, KH, NI], bf16, tag="y")
            for k in range(KH):
                nc.vector.tensor_scalar(
                    y[:, k, :], Gbm[:, k, :], Hcol[:, jt, k:k + 1], 1.0,
                    MULT, MAX,
                )
            # mask multiply, split for DVE/Pool balance: DVE takes heads 0+1
            # (one broadcast op); Pool takes head 3 and every other j-chunk
            # also head 2, except the final two j-chunks (all-DVE so the
            # slower Pool ops never gate the drain into the epilogue).
            nc.vector.tensor_mul(
                y[:, 0:2, :], y[:, 0:2, :],
                mTs[jt][:, None, :].broadcast_to((P, 2, NI)))
            tail = jt >= JT - 3
            k2eng = nc.gpsimd if (jt % 2 == 0 and not tail) else nc.vector
            k3eng = nc.vector if tail else nc.gpsimd
            k2eng.tensor_mul(y[:, 2, :], y[:, 2, :], mTs[jt][:, :])
            k3eng.tensor_mul(y[:, 3, :], y[:, 3, :], mTs[jt][:, :])
            for k in range(KH):
                for h in range(NIB):
                    nc.tensor.matmul(
                        pv[k][h][:],
                        whf[jt][:, k, :],
                        y[:, k, h * IBS:(h + 1) * IBS],
                        start=(jt == 0),
                        stop=(jt == JT - 1),
                    )

        # epilogue: raw [33, i] accumulators (numerators + denominator row)
        # through SBUF (DMA cannot read PSUM), copies split ACT/DVE; the
        # host divides/un-transposes
        for h in range(NIB):
            otT = work.tile([DH + 1, KH, IBS], f32, tag="otT", bufs=2,
                            name=f"otT{h}")
            for k in range(KH):
                if k % 2 == 0:
                    nc.scalar.copy(otT[:, k, :], pv[k][h][:])
                else:
                    nc.vector.tensor_copy(otT[:, k, :], pv[k][h][:])
            dq = nc.sync if h == 0 else nc.scalar
            dq.dma_start(oaux_d[h].rearrange("k d i -> d k i"), otT[:])

    nc.compile()
    return nc


def _host_prep(H, A, W, a_src, a_dst):
    """Build the 8 per-core input maps (layout/dtype prep only)."""
    import ml_dtypes
    bf = ml_dtypes.bfloat16

    Ssrc = np.zeros((FIN, KH), np.float32)
    Sdst = np.zeros((FIN, KH), np.float32)
    for k in range(KH):
        Ssrc[k * DH:(k + 1) * DH, k] = a_src[k]
        Sdst[k * DH:(k + 1) * DH, k] = a_dst[k]
    WS = (W.astype(np.float32) @ Ssrc)  # fold W into the score vectors
    WD = (W.astype(np.float32) @ Sdst)

    sel = np.zeros((KH, KH * P), np.float32)
    for k in range(KH):
        sel[k, k * P:(k + 1) * P] = 1.0
    sel = sel.astype(bf)
    wu_host = np.ones((P, 8 + P), np.float32)
    wu_host[:, 8:] = np.eye(P, dtype=np.float32)
    wu_host = wu_host.astype(bf)

    maskB = (A > 0)  # [B, N, N] bool

    in_maps = []
    for c in range(8):
        b, half = divmod(c, 2)
        i0 = half * NI
        HbT = np.roll(H[b], -i0, axis=0).T
        maskT = np.ascontiguousarray(
            np.roll(maskB[b, i0:i0 + NI, :], -i0, axis=1).T
        ).astype(bf)
        cpackA = np.concatenate([WS, WD, HbT[:, 0:NI]], axis=1).astype(bf)
        cpackB = np.concatenate(
            [W.astype(np.float32), np.eye(P, dtype=np.float32), HbT[:, NI:]],
            axis=1,
        ).astype(bf)
        in_maps.append({
            "maskT": maskT,
            "cpackA": np.ascontiguousarray(cpackA),
            "cpackB": np.ascontiguousarray(cpackB),
            "wu": wu_host,
            "sel": sel,
        })
    return in_maps


def kernel(H, A, W, a_src, a_dst, _want_results=False, _trace=False):
    H = np.asarray(H); A = np.asarray(A); W = np.asarray(W)
    a_src = np.asarray(a_src); a_dst = np.asarray(a_dst)

    if "nc" not in _CACHE:
        _CACHE["nc"] = _build_program()
    nc = _CACHE["nc"]

    in_maps = _host_prep(H, A, W, a_src, a_dst)
    res = run_bass_kernel_spmd(nc, in_maps, list(range(8)), trace=_trace)

    out = np.empty((B, N, KH * DH), np.float32)
    for c in range(8):
        b, half = divmod(c, 2)
        i0 = half * NI
        aux = res.results[c]["oaux"]  # [NIB, KH, DH+1, IBS]
        slab = aux[:, :, 0:DH, :] / aux[:, :, DH:DH + 1, :]
        # [h, k, d, i] -> rows (h*IBS + i), cols (k*DH + d)
        out[b, i0:i0 + NI, :] = (
            slab.transpose(0, 3, 1, 2).reshape(NI, KH * DH)
        )
    if _want_results:
        return out, res
    return out
